# revision 9
# baseline (speedup 1.0000x reference)
"""Trainium2 Bass kernel for nn_CustomModel_74586402063130.

Model: logits = GRU(E[words] + midis@Wm, mask=words!=0) @ Wo
Shapes: B=32, T=256, V=10000, E_DIM=300, M_DIM=128, H=256.

Strategy (8 cores): replicate the embedding+GRU on every core (the
recurrence is latency-bound, not FLOP-bound), shard the output Dense
over the vocab dimension (1250 columns per core). No collectives.

Per-core dataflow (all matmuls bf16, fp32 PSUM, transposed layouts —
feature dims on partitions, tokens on the free axis, token col order
t*32+b):
  xT   [302, 8192]  = gathered E rows (indirect DMA, then PE-matmul
                      transpose into PSUM) + Wm.T@midisT accumulated in
                      the same PSUM tile (+ ones row for folded biases,
                      + (1-mask) row)
  gxT  [768, 8192]  = Wx'.T @ xT   (z-gate columns negated, mask bias and
                      bx/bm folded in via the extra xT rows)
  recurrence over t: PSUM r = id@gx_r[t] + U_r.T h   (emitted first)
                     PSUM n = U_n.T h
                     PSUM z = id@gx_z[t] + U_z.T h   (z negated -> s=1-z)
                     r = sigmoid(PSUM_r); s = sigmoid(PSUM_z)
                     q = PSUM_n * r; p = q + gx_n[t]; n = tanh(p)
                     v = s*h; w = h - v   (off critical path, during tanh)
                     h' = w + s*n        (pad steps: s ~= 0 keeps h)
  logits[t-block]   = seq.T-as-weights @ Wo-slice, PSUM->SBUF(lt)->DRAM,
                      pipelined into the recurrence; out is t-major
                      [T, B, VSH] so each DMA is one contiguous run.
All elementwise/copy side work is spread across Act/DVE/Pool so no
engine sequencer blocks the serial recurrence chain.
"""
import numpy as np
import ml_dtypes
from contextlib import ExitStack

B, T, V, E_DIM, M_DIM, H = 32, 256, 10000, 300, 128, 256
NBT = B * T              # 8192 tokens
NCORES = 8
VSH = V // NCORES        # 1250 vocab columns per core
EP = 384                 # padded embedding row (bf16: 768B, %256 for gather)
BIG = 30.0               # sigmoid(-30) ~ 1e-13: masked steps keep h exactly
CG = 512                 # token col-group size for the x/gx pipeline
NCG = NBT // CG          # 16 col groups
KC2 = E_DIM - 256        # 44 valid embedding rows in chunk 2
KCH = [128, 128, KC2 + 2]  # chunk K sizes for the gx matmul (+ones/mask rows)

bf16 = ml_dtypes.bfloat16


def _host_prep(words, midis, E, Wm, bm, Wx, bx, U, bu, Wo, bo):
    """Numpy-side weight/index preparation (O(V*E) transforms only)."""
    f32 = np.float32
    words = np.asarray(words)
    mask = (words != 0)

    # token col order: col = t*32 + b
    iw = np.ascontiguousarray(words.T).reshape(-1)          # [8192] t-major
    # indirect-DMA gather: tile i gathers rows iw[i*128 + p] into partition p
    idx32 = np.ascontiguousarray(
        iw.reshape(NBT // 128, 128).T).astype(np.int32)     # [128, 64]

    e_pad = np.zeros((V, EP), dtype=bf16)
    e_pad[:, :E_DIM] = np.asarray(E, f32).astype(bf16)

    midisT = np.ascontiguousarray(
        np.asarray(midis, f32).transpose(1, 0, 2).reshape(NBT, M_DIM).T
    ).astype(bf16)                                          # [128, 8192]

    onesmask = np.empty((2, NBT), dtype=bf16)
    onesmask[0] = 1.0
    onesmask[1] = (1.0 - np.ascontiguousarray(mask.T).reshape(-1)).astype(bf16)

    wm = np.zeros((M_DIM, 3 * 128), dtype=bf16)             # chunk-major M
    wm[:, :E_DIM] = np.asarray(Wm, f32).astype(bf16)        # cols 300..383: 0

    # Wx' rows: 0..299 = Wx (z cols negated), 300 = bias row, 301 = mask row
    wxp_f = np.zeros((3 * 128, 3 * H), dtype=f32)
    wxp_f[:E_DIM] = np.asarray(Wx, f32)
    wxp_f[E_DIM] = np.asarray(bm, f32) @ np.asarray(Wx, f32) + np.asarray(bx, f32)
    wxp_f[:, :H] *= -1.0                                    # negate z columns
    wxp_f[E_DIM + 1, :H] = -BIG                             # mask row weight
    # also fold bu_z, bu_r into the per-step bias (added every step)
    buf = np.asarray(bu, f32)
    wxp_f[E_DIM, :H] += -buf[:H]
    wxp_f[E_DIM, H:2 * H] += buf[H:2 * H]
    # store chunk-major: [128, 3*768], chunk c = rows c*128..c*128+127
    wxp = np.ascontiguousarray(
        wxp_f.reshape(3, 128, 3 * H).transpose(1, 0, 2).reshape(128, 3 * 3 * H)
    ).astype(bf16)

    up_f = np.asarray(U, f32).copy()
    up_f[:, :H] = -up_f[:, :H]
    up = np.ascontiguousarray(
        up_f.reshape(2, 128, 3 * H).transpose(1, 0, 2).reshape(128, 2 * 3 * H)
    ).astype(bf16)                                          # [128, 2*768]

    bun = np.asarray(bu, f32)[2 * H:]                       # [256] n-gate bias
    bun2 = np.ascontiguousarray(bun.reshape(2, 128).T).astype(f32)  # [128, 2]

    ident = np.eye(128, dtype=bf16)

    wo_f = np.asarray(Wo, f32)                              # [256, V]
    wo_cores = []
    for c in range(NCORES):
        sl = wo_f[:, c * VSH:(c + 1) * VSH]                 # [256, 1250]
        wo_cores.append(np.ascontiguousarray(
            sl.reshape(2, 128, VSH).transpose(1, 0, 2).reshape(128, 2 * VSH)
        ).astype(bf16))

    bo_f = np.asarray(bo, f32)
    return dict(idx32=idx32, e_pad=e_pad, midisT=midisT, onesmask=onesmask,
                wm=wm, wxp=wxp, up=up, bun2=bun2, ident=ident,
                wo_cores=wo_cores, bo_f=bo_f,
                has_bun=bool(np.any(np.asarray(bu, f32)[2 * H:])),
                has_bo=bool(np.any(bo_f)))


def _apply_tile_patch():
    """This container's walrus rejects >1 semaphore wait on a Drain
    instruction; Tile's kernel-tail drain aggregates one wait per active
    sem lane onto a single Drain. Split them across a chain of Drains."""
    import concourse.mybir as mybir
    import concourse.tile as tile
    from concourse.vector_clock import ScopedClock

    if getattr(tile.TileContext, "_drain_split_patched", False):
        return

    def _patched(self, tick_clock, wait_clock):
        drain_inst = self.nc.sync.drain()
        wait_clock.add_sem_waits(
            drain_inst.ins, ScopedClock({None: tick_clock.global_clock})
        )
        inst = drain_inst.ins
        si = inst.sync_info
        if si is not None and len(si.on_wait) > 1:
            waits = list(si.on_wait)
            si.on_wait = waits[:1]
            inst.sync_info = si
            for w in waits[1:]:
                d2 = self.nc.sync.drain()
                si2 = d2.ins.sync_info or mybir.SyncInfo(on_wait=[], on_update=[])
                si2.on_wait = [w]
                d2.ins.sync_info = si2
        self.nc.all_engine_barrier()
        assert self.sems is not None
        popped = self.nc._tile_sem_poison_stack.pop()
        assert popped is self._sem_poison
        self.nc.clear_and_free_semaphores(list(self.sems.allocated().values()))
        self.nc.all_engine_barrier()

    tile.TileContext._drain_and_barrier = _patched
    tile.TileContext._drain_split_patched = True


def _split_multiwaits(nc):
    """This container's walrus codegen accepts at most ONE semaphore wait
    per instruction. Hoist extra waits onto NoOps inserted just before the
    offending instruction on the same engine (engine streams are in-order,
    so waiting earlier on the same queue is equivalent). The wait with the
    least slack at this program point (the producer that completes last —
    the real data dependency) stays on the instruction itself so it is
    evaluated in the engine wait queue instead of blocking the sequencer;
    already-satisfied waits ride the NoOps cheaply."""
    import concourse.mybir as mybir

    ctr = [0]
    for fn in nc.m.functions:
        for bb in fn.blocks:
            changed = False
            new_insts = []
            counts = {}
            for inst in bb.instructions:
                si = inst.sync_info
                if si is not None and len(si.on_wait) > 1:
                    waits = list(si.on_wait)
                    keep = min(
                        range(len(waits)),
                        key=lambda i: counts.get(waits[i].ant_name, 0)
                        - waits[i].wait_value)
                    for i, w in enumerate(waits):
                        if i == keep:
                            continue
                        nop = mybir.InstNoOp(
                            name=f"I-mwsplit-{ctr[0]}", ins=[], outs=[])
                        ctr[0] += 1
                        nop.engine = inst.engine
                        nop.sync_info = mybir.SyncInfo(
                            on_wait=[w], on_update=[])
                        new_insts.append(nop)
                    si.on_wait = [waits[keep]]
                    inst.sync_info = si
                    changed = True
                new_insts.append(inst)
                if si is not None:
                    for u in si.on_update:
                        counts[u.ant_name] = (
                            counts.get(u.ant_name, 0) + u.update_value)
            if changed:
                bb.instructions = new_insts


def build_nc(has_bun=False, has_bo=False, debug=False, reps=1):
    import concourse.bass as bass
    import concourse.mybir as mybir
    import concourse.tile as tile

    _apply_tile_patch()
    dt = mybir.dt
    nc = bass.Bass()

    e_d = nc.declare_dram_parameter("e_pad", [V, EP], dt.bfloat16, isOutput=False)
    idx_d = nc.declare_dram_parameter("idx32", [128, NBT // 128], dt.int32, isOutput=False)
    mid_d = nc.declare_dram_parameter("midisT", [M_DIM, NBT], dt.bfloat16, isOutput=False)
    om_d = nc.declare_dram_parameter("onesmask", [2, NBT], dt.bfloat16, isOutput=False)
    wm_d = nc.declare_dram_parameter("wm", [M_DIM, 3 * 128], dt.bfloat16, isOutput=False)
    wxp_d = nc.declare_dram_parameter("wxp", [128, 9 * H], dt.bfloat16, isOutput=False)
    up_d = nc.declare_dram_parameter("up", [128, 6 * H], dt.bfloat16, isOutput=False)
    bun_d = nc.declare_dram_parameter("bun2", [128, 2], dt.float32, isOutput=False)
    id_d = nc.declare_dram_parameter("ident", [128, 128], dt.bfloat16, isOutput=False)
    wo_d = nc.declare_dram_parameter("wo", [128, 2 * VSH], dt.bfloat16, isOutput=False)
    bo_d = nc.declare_dram_parameter("bo_b", [128, VSH], dt.float32, isOutput=False)
    # t-major output: kernel() transposes to [B, T, VSH] on the host.
    out_d = nc.declare_dram_parameter("out", [T, B, VSH], dt.float32, isOutput=True)
    if debug:
        gx_dbg = nc.declare_dram_parameter("gx_dbg", [128, 6 * NBT], dt.bfloat16, isOutput=True)
        seq_dbg = nc.declare_dram_parameter("seq_dbg", [128, 2 * T * 32], dt.bfloat16, isOutput=True)
        xe_dbg = nc.declare_dram_parameter("xe_dbg", [128, 3 * CG], dt.bfloat16, isOutput=True)

    with tile.TileContext(nc) as tc, ExitStack() as ctx:
        singles = ctx.enter_context(tc.tile_pool(name="singles", bufs=1))
        big = ctx.enter_context(tc.tile_pool(name="big", bufs=1))
        xep = ctx.enter_context(tc.tile_pool(name="xep", bufs=3))
        gpool = ctx.enter_context(tc.tile_pool(name="gpool", bufs=5))
        midp = ctx.enter_context(tc.tile_pool(name="midp", bufs=2))
        work = ctx.enter_context(tc.tile_pool(name="work", bufs=3))
        loutp = ctx.enter_context(tc.tile_pool(name="lout", bufs=2))
        prec = ctx.enter_context(tc.tile_pool(name="prec", bufs=2, space="PSUM"))
        pxep = ctx.enter_context(tc.tile_pool(name="pxe", bufs=2, space="PSUM"))
        pgxp = pxep          # x-transpose and gx PSUM lifetimes don't overlap
        plogp = ctx.enter_context(tc.tile_pool(name="plog", bufs=2, space="PSUM"))

        f32, b16 = dt.float32, dt.bfloat16

        # ---- resident tensors ----
        idx_s = singles.tile([128, NBT // 128], dt.int32)
        nc.sync.dma_start(out=idx_s[:], in_=idx_d[:])
        wm_s = singles.tile([M_DIM, 3 * 128], b16)
        nc.sync.dma_start(out=wm_s[:], in_=wm_d[:])
        wxp_s = singles.tile([128, 9 * H], b16)
        nc.sync.dma_start(out=wxp_s[:], in_=wxp_d[:])
        up_s = singles.tile([128, 6 * H], b16)
        nc.sync.dma_start(out=up_s[:], in_=up_d[:])
        id_s = singles.tile([128, 128], b16)
        nc.sync.dma_start(out=id_s[:], in_=id_d[:])
        wo_s = singles.tile([128, 2 * VSH], b16)
        nc.sync.dma_start(out=wo_s[:], in_=wo_d[:])
        bun_s = singles.tile([128, 2], f32)
        if has_bun:
            nc.sync.dma_start(out=bun_s[:], in_=bun_d[:])
        bo_s = singles.tile([128, VSH], f32)
        if has_bo:
            nc.sync.dma_start(out=bo_s[:], in_=bo_d[:])

        gxT = big.tile([128, 6, NBT], b16)                  # 96KB/part
        seqT = big.tile([128, 2, T, 32], b16)               # 32KB/part
        h0 = singles.tile([128, 2, 32], b16)
        nc.vector.memset(h0[:], 0.0)

        # per-colgroup live tiles, keyed by cg (pieces span several steps)
        cg_state = {}

        # Deferred PSUM->SBUF copies: the profile shows a 500-700ns copy
        # emitted between two chain ops stalls the serial recurrence by its
        # full duration.  All copies go through this FIFO instead and are
        # flushed at a fixed point of each step (at most one per engine per
        # step), where they overlap the next step's PE segment.
        pending = []

        def flush_copies(nact=1, ndve=1):
            for _ in range(nact):
                if pending:
                    pending.pop(0)(nc.scalar)
            for _ in range(ndve):
                if pending:
                    pending.pop(0)(nc.vector)

        def cg_piece(cg, k, defer=None):
            """Piece k (0..15) of the x/gx pipeline for col-group cg.
            PE/DMA work is emitted inline; PSUM->SBUF copies go through
            `pending` (flushed one-per-engine-per-step) so they fill engine
            idle windows instead of blocking the chain."""
            c0 = cg * CG
            st = cg_state.setdefault(cg, {})
            if k == 0:
                st["g"] = [gpool.tile([128, EP], b16, tag="gath", name="g%d" % i)
                           for i in range(4)]
                for gt in range(2):
                    nc.gpsimd.indirect_dma_start(
                        out=st["g"][gt][:],
                        out_offset=None,
                        in_=e_d[:],
                        in_offset=bass.IndirectOffsetOnAxis(
                            ap=idx_s[:, cg * 4 + gt: cg * 4 + gt + 1], axis=0),
                    )
            elif k == 1:
                for gt in range(2, 4):
                    nc.gpsimd.indirect_dma_start(
                        out=st["g"][gt][:],
                        out_offset=None,
                        in_=e_d[:],
                        in_offset=bass.IndirectOffsetOnAxis(
                            ap=idx_s[:, cg * 4 + gt: cg * 4 + gt + 1], axis=0),
                    )
                st["mt"] = midp.tile([M_DIM, CG], b16, tag="mid", name="mt")
                nc.sync.dma_start(out=st["mt"][:], in_=mid_d[:, c0:c0 + CG])
                st["xe"] = xep.tile([128, 3, CG], b16, tag="xe", name="xe")
                # ones + mask rows (chunk 2, rows 44/45)
                nc.sync.dma_start(out=st["xe"][KC2:KC2 + 2, 2, :],
                                  in_=om_d[:, c0:c0 + CG])
            elif k in (4, 6, 8):
                c = (k - 4) // 2
                # transpose the gathered E rows into feature-major via PE and
                # accumulate the midi projection into the same PSUM. Each
                # 128-col block is a well-formed group: transpose (start) then
                # a midis matmul piece over the same region (stop) — wm's
                # zero-padded columns make every chunk a full 128 rows.
                px = pxep.tile([128, CG], f32, tag="paux", name="px")
                st["px%d" % c] = px
                for gt in range(4):
                    blk = slice(gt * 128, (gt + 1) * 128)
                    nc.tensor.matmul(
                        px[:, blk],
                        st["g"][gt][:, c * 128:(c + 1) * 128],
                        id_s[:],
                        start=True, stop=False)
                    nc.tensor.matmul(
                        px[:, blk], wm_s[:, c * 128:(c + 1) * 128],
                        st["mt"][:, blk], start=False, stop=True)
                mc = [128, 128, KC2][c]
                # copy PSUM -> xe (bf16); chunk 2 only its 44 valid rows
                # (GPSIMD cannot access PSUM: copies live on Act/DVE)
                xe = st["xe"]

                def copy_xe(eng, xe=xe, px=px, mc=mc, c=c):
                    if eng is nc.scalar:
                        eng.copy(xe[:mc, c, :], px[:mc, :])
                    else:
                        eng.tensor_copy(xe[:mc, c, :], px[:mc, :])
                (pending.append(copy_xe) if defer is not None else copy_xe(
                    [nc.scalar, nc.vector, nc.scalar][c]))
            elif 10 <= k <= 15:
                m = k - 10
                xe = st["xe"]
                pg = pgxp.tile([128, CG], f32, tag="paux", name="pg")
                for c in range(3):
                    nc.tensor.matmul(
                        pg[:, :],
                        wxp_s[:KCH[c], c * 3 * H + m * 128: c * 3 * H + m * 128 + 128],
                        xe[:KCH[c], c, :],
                        start=(c == 0), stop=(c == 2))
                def copy_gx(eng, pg=pg, m=m, c0=c0):
                    if eng is nc.scalar:
                        eng.copy(gxT[:, m, c0:c0 + CG], pg[:, :])
                    else:
                        eng.tensor_copy(gxT[:, m, c0:c0 + CG], pg[:, :])
                (pending.append(copy_gx) if defer is not None else copy_gx(
                    [nc.vector, nc.scalar][m % 2]))
                if k == 15:
                    if debug and cg == 0:
                        nc.sync.dma_start(out=xe_dbg[:], in_=st["xe"][:, :, :])
                    cg_state.pop(cg, None)

        NS = [512, 512, VSH - 1024]
        lt_cur = [None]

        def emit_logits_piece(tb, ns, defer=None):
            """One N-split of the output GEMM for t-block tb (4 steps)."""
            t0 = tb * 4
            if ns == 0 and tb % 2 == 0:
                lt_cur[0] = loutp.tile([128, 2, VSH], f32, tag="lt", name="lt")
            lt = lt_cur[0]
            base = seqT[:, 0, 0, :]
            n0 = ns * 512
            pl = plogp.tile([128, 512], f32, tag="plog", name="pl")
            for cc in range(2):
                lhsT = bass.AP(
                    tensor=base.tensor,
                    offset=base.offset + (cc * T * 32 + t0 * 32),
                    ap=[base.ap[0], [1, 128]])
                nc.tensor.matmul(pl[:, :NS[ns]], lhsT,
                                 wo_s[:, cc * VSH + n0: cc * VSH + n0 + NS[ns]],
                                 start=(cc == 0), stop=(cc == 1))
            def copy_lt(eng, lt=lt, pl=pl, tb=tb, ns=ns, n0=n0):
                if eng is nc.scalar:
                    eng.copy(lt[:, tb % 2, n0:n0 + NS[ns]], pl[:, :NS[ns]])
                else:
                    eng.tensor_copy(lt[:, tb % 2, n0:n0 + NS[ns]], pl[:, :NS[ns]])
                if ns == 2 and tb % 2 == 1:
                    if has_bo:
                        for j in range(2):
                            nc.vector.tensor_add(lt[:, j, :], lt[:, j, :], bo_s[:])
                    # one DMA for 8 timesteps: out is t-major so SBUF
                    # partition p = dti*32+b maps to a uniform DRAM stride.
                    t0p = (tb - 1) * 4
                    dst = bass.AP(
                        tensor=out_d[:].tensor,
                        offset=t0p * B * VSH,
                        ap=[[VSH, 128], [4 * B * VSH, 2], [1, VSH]])
                    nc.sync.dma_start(out=dst, in_=lt[:, :, :])
            (pending.append(copy_lt) if defer is not None else copy_lt(
                [nc.scalar, nc.vector, nc.vector][ns]))

        psum_next = {}

        def inject_idgx(t):
            """Pre-write id@gx for step t into fresh PSUM tiles one step
            early (groups left open; step t's U matmuls continue them), so
            the injection is off the h'->sigmoid critical PE segment."""
            ts32 = slice(t * 32, (t + 1) * 32)
            prn = prec.tile([128, 4, 32], f32, tag="prn", name="prn")
            pz = prec.tile([128, 2, 32], f32, tag="pz", name="pz")
            nc.tensor.matmul(prn[:, 0:2, :], id_s[:], gxT[:, 2:4, ts32],
                             start=True, stop=False, skip_group_check=True)
            nc.tensor.matmul(pz[:, :, :], id_s[:], gxT[:, 0:2, ts32],
                             start=True, stop=False, skip_group_check=True)
            psum_next[t] = (prn, pz)

        def emit_step(t):
            """One GRU step; PE aux (logits/colgroup) is emitted after the
            recurrence matmuls so it fills the sigmoid/tanh windows."""
            ts32 = slice(t * 32, (t + 1) * 32)
            hprev = h0[:] if t == 0 else seqT[:, :, t - 1, :]
            # r and n share one PSUM tile: per-tile dep tracking then lets
            # q's PE wait be subsumed by its wait on sigmoid(r).
            prn, pz = psum_next.pop(t)
            pr = prn[:, 0:2, :]
            pn = prn[:, 2:4, :]
            # r first: its sigmoid heads the serial chain
            for j in range(2):
                for kc in range(2):
                    nc.tensor.matmul(
                        prn[:, j, :],
                        up_s[:, kc * 3 * H + (2 + j) * 128: kc * 3 * H + (3 + j) * 128],
                        hprev[:, kc, :],
                        start=False, stop=(kc == 1), skip_group_check=True)
            for j in range(2):      # n
                for kc in range(2):
                    nc.tensor.matmul(
                        prn[:, 2 + j, :],
                        up_s[:, kc * 3 * H + (4 + j) * 128: kc * 3 * H + (5 + j) * 128],
                        hprev[:, kc, :],
                        start=(kc == 0), stop=(kc == 1), skip_group_check=True)
            for j in range(2):      # z
                for kc in range(2):
                    nc.tensor.matmul(
                        pz[:, j, :],
                        up_s[:, kc * 3 * H + j * 128: kc * 3 * H + (j + 1) * 128],
                        hprev[:, kc, :],
                        start=False, stop=(kc == 1), skip_group_check=True)
            if t + 1 < T:
                inject_idgx(t + 1)
            if lpieces:
                emit_logits_piece(*lpieces.pop(0), defer=True)

            r_t = work.tile([128, 2, 32], b16, tag="r")
            nc.scalar.activation(r_t[:], pr, mybir.ActivationFunctionType.Sigmoid)
            s_t = work.tile([128, 2, 32], b16, tag="s")
            nc.scalar.activation(s_t[:], pz[:], mybir.ActivationFunctionType.Sigmoid)
            # z-path off the chain, on Pool: v = s*h, w = h - s*h run during
            # the q/p/tanh window.  h' = w + s*n keeps h' == h exactly on
            # masked steps (s ~= 0 makes v round to 0 and w to h in bf16).
            v = work.tile([128, 2, 32], b16, tag="v")
            nc.gpsimd.tensor_mul(v[:], s_t[:], hprev)
            w = work.tile([128, 2, 32], b16, tag="w")
            nc.gpsimd.tensor_sub(w[:], hprev, v[:])
            q = work.tile([128, 2, 32], b16, tag="q")
            if has_bun:
                for j in range(2):
                    nc.vector.scalar_tensor_tensor(
                        q[:, j, :], prn[:, 2 + j, :], bun_s[:, j:j + 1],
                        r_t[:, j, :],
                        op0=mybir.AluOpType.add, op1=mybir.AluOpType.mult)
            else:
                nc.vector.tensor_mul(q[:], pn, r_t[:])
            p = work.tile([128, 2, 32], b16, tag="p")
            nc.vector.tensor_add(p[:], q[:], gxT[:, 4:6, ts32])
            n_t = work.tile([128, 2, 32], b16, tag="n")
            nc.scalar.activation(n_t[:], p[:], mybir.ActivationFunctionType.Tanh)
            e2 = work.tile([128, 2, 32], b16, tag="e")
            nc.vector.tensor_mul(e2[:], s_t[:], n_t[:])
            nc.vector.tensor_add(seqT[:, :, t, :], w[:], e2[:])

            # copy slots + colgroup piece at the tail of the step: gathers
            # (Pool) land after v/w, PE pieces overlap the next chain.
            flush_copies()
            cg = t // 16 + 2
            if cg < NCG:
                cg_piece(cg, t % 16, defer=True)

        # ---- prologue: first two col groups, then the recurrence ----
        lpieces = []
        # `reps` repeats the whole computation back-to-back inside the NEFF
        # (timing runs only; WAR deps on gxT/seqT serialize the repetitions).
        for _rep in range(reps):
            lpieces.clear()
            pending.clear()
            for k in range(16):
                cg_piece(0, k)
            for k in range(16):
                cg_piece(1, k)
            inject_idgx(0)
            for t in range(T):
                emit_step(t)
                if t % 4 == 3:
                    tb = t // 4
                    lpieces.extend([(tb, 0), (tb, 1), (tb, 2)])
            i = 0
            while pending:
                pending.pop(0)(nc.scalar if i % 2 else nc.vector)
                i += 1
            while lpieces:
                emit_logits_piece(*lpieces.pop(0))
        if debug:
            nc.sync.dma_start(out=gx_dbg[:], in_=gxT[:, :, :])
            nc.sync.dma_start(out=seq_dbg[:], in_=seqT[:, :, :, :])

    _split_multiwaits(nc)
    return nc


_BUILD_CACHE = {}


def _get_built(has_bun, has_bo):
    key = (has_bun, has_bo)
    if key not in _BUILD_CACHE:
        _BUILD_CACHE[key] = build_nc(has_bun, has_bo)
    return _BUILD_CACHE[key]


def make_in_maps(prep):
    maps = []
    for c in range(NCORES):
        m = dict(e_pad=prep["e_pad"], idx32=prep["idx32"],
                 midisT=prep["midisT"], onesmask=prep["onesmask"],
                 wm=prep["wm"], wxp=prep["wxp"], up=prep["up"],
                 bun2=prep["bun2"], ident=prep["ident"],
                 wo=prep["wo_cores"][c],
                 bo_b=np.broadcast_to(
                     prep["bo_f"][c * VSH:(c + 1) * VSH].astype(np.float32),
                     (128, VSH)).copy())
        maps.append(m)
    return maps


_EXEC_CACHE = {}


def _get_executor(nc):
    """Build (once) a reusable sharded PJRT executable for `nc` across the
    8 cores. run_bass_kernel_spmd's axon path re-jits on every call; caching
    the jitted function makes repeated kernel() calls cheap."""
    key = id(nc)
    if key in _EXEC_CACHE:
        return _EXEC_CACHE[key]
    import jax
    from jax.sharding import Mesh, PartitionSpec
    from jax.experimental.shard_map import shard_map
    from concourse import bass2jax
    import concourse.mybir as mybir

    bass2jax.install_neuronx_cc_hook()
    in_names, out_names, out_avals, zero_outs = [], [], [], []
    for alloc in nc.m.functions[0].allocations:
        if not isinstance(alloc, mybir.MemoryLocationSet):
            continue
        name = alloc.memorylocations[0].name
        if alloc.kind == "ExternalInput":
            if nc.partition_id_tensor is None or name != nc.partition_id_tensor.name:
                in_names.append(name)
        elif alloc.kind == "ExternalOutput":
            shape = tuple(alloc.tensor_shape)
            dtype = mybir.dt.np(alloc.dtype)
            out_names.append(name)
            out_avals.append(jax.core.ShapedArray(shape, dtype))
            zero_outs.append(np.zeros(shape, dtype))
    n_params = len(in_names)

    partition_name = (nc.partition_id_tensor.name
                      if nc.partition_id_tensor else None)
    bind_in_names = list(in_names) + list(out_names)
    if partition_name is not None:
        bind_in_names.append(partition_name)

    def _body(*args):
        operands = list(args)
        if partition_name is not None:
            operands.append(bass2jax.partition_id_tensor())
        outs = bass2jax._bass_exec_p.bind(
            *operands,
            out_avals=tuple(out_avals),
            in_names=tuple(bind_in_names),
            out_names=tuple(out_names),
            lowering_input_output_aliases=(),
            sim_require_finite=True,
            sim_require_nnan=True,
            nc=nc)
        return tuple(outs)

    devices = jax.devices()[:NCORES]
    mesh = Mesh(np.asarray(devices), ("core",))
    in_specs = (PartitionSpec("core"),) * (n_params + len(out_avals))
    out_specs = (PartitionSpec("core"),) * len(out_avals)
    sharded = jax.jit(
        shard_map(_body, mesh=mesh, in_specs=in_specs, out_specs=out_specs,
                  check_rep=False),
        keep_unused=True)

    def run(in_maps):
        per_core = [[np.asarray(m[name]) for name in in_names] for m in in_maps]
        concat_in = [np.concatenate([per_core[c][i] for c in range(NCORES)], axis=0)
                     for i in range(n_params)]
        concat_zeros = [np.zeros((NCORES * z.shape[0], *z.shape[1:]), z.dtype)
                        for z in zero_outs]
        outs = sharded(*concat_in, *concat_zeros)
        jax.block_until_ready(outs)
        return [
            {name: np.asarray(outs[i]).reshape(NCORES, *out_avals[i].shape)[c]
             for i, name in enumerate(out_names)}
            for c in range(NCORES)
        ]

    _EXEC_CACHE[key] = run
    return run


def kernel(words, midis, E, Wm, bm, Wx, bx, U, bu, Wo, bo):
    prep = _host_prep(words, midis, E, Wm, bm, Wx, bx, U, bu, Wo, bo)
    nc = _get_built(prep["has_bun"], prep["has_bo"])
    results = _get_executor(nc)(make_in_maps(prep))
    # out is t-major [T, B, VSH] per core; stitch vocab then transpose.
    full = np.concatenate([r["out"] for r in results], axis=2)
    return np.ascontiguousarray(full.transpose(1, 0, 2))

